# revision 1
# baseline (speedup 1.0000x reference)
"""Trainium2 Bass kernel for a 16-head attention block (d_model=1024, seq=4096).

Sharding: tensor-parallel over heads. Each of the 8 cores computes QKV
projections, RMSNorm(q,k), full softmax(QK^T)V attention for its 2 heads,
and a partial O-projection (its heads' slice of the contraction). The host
sums the 8 partial outputs and adds the output bias (the all-reduce of the
TP decomposition, done at unshard time).

Per-core dataflow (all matmuls fp32r, contraction always 128):
  phase 1: qkv[s,f] = xT_aug^T @ WqkvT_aug (bias via augmented ones row),
           RMSNorm over d_head on the [s_part, d_free] layout,
           PE-transpose q_hat/k_hat per head into [d(pad 128), s] tiles,
           V' = [V | 1] chunks for the fused denominator.
  phase 2: per (head, q-tile 512): S^T[k,q] blocks via k_hatT.T @ q_hatT,
           exp on ACT, z'^T[65,q] += V'^T @ probs accumulated over k-blocks
           (row 64 = softmax denominators), reciprocal + ones-row broadcast
           matmul + multiply -> z_nT[d_local 128, s].
  phase 3: out[s,dm] partial = z_nT.T @ WoT slice, PSUM->HBM direct.
"""

import numpy as np
from contextlib import ExitStack

import concourse.bass as bass
import concourse.tile as tile
from concourse import mybir
from concourse.masks import make_identity

F32 = mybir.dt.float32
F32R = mybir.dt.float32r
AF = mybir.ActivationFunctionType
ALU = mybir.AluOpType

D_MODEL = 1024
SEQ = 4096
N_HEADS = 16
D_HEAD = 64
N_CORES = 8
HEADS_LOCAL = 2
P = 128
F_LOCAL = 3 * HEADS_LOCAL * D_HEAD       # 384: [q0|q1|k0|k1|v0|v1]
DM_CHUNKS = D_MODEL // P                 # 8
DM_AUG = D_MODEL + P                     # 1152 rows: x^T plus ones-row block
SB = SEQ // P                            # 32 s-blocks
QT = 8                                   # q-tiles of 512
QW = SEQ // QT                           # 512
KB = SEQ // P                            # 32 k-blocks
EPS = 1e-6


class _SplitWaitTileContext(tile.TileContext):
    """TileContext whose exit drain splits its semaphore waits across
    single-wait sync nops: this walrus rejects CTRL instructions carrying
    more than one sync-wait command."""

    MAX_CTRL_WAITS = 1

    def _drain_and_barrier(self, tick_clock, wait_clock):
        from concourse.vector_clock import ScopedClock

        nc = self.nc
        carrier = nc.sync.nop(nofuse=True)
        wait_clock.add_sem_waits(
            carrier.ins, ScopedClock({None: tick_clock.global_clock})
        )
        si = carrier.ins.sync_info
        waits = list(si.on_wait) if si is not None and si.on_wait else []
        if len(waits) > self.MAX_CTRL_WAITS:
            sic = type(si)
            carrier.ins.sync_info = sic(
                on_wait=waits[: self.MAX_CTRL_WAITS], on_update=si.on_update
            )
            for i in range(self.MAX_CTRL_WAITS, len(waits), self.MAX_CTRL_WAITS):
                chunk = waits[i : i + self.MAX_CTRL_WAITS]
                w2 = nc.sync.nop(nofuse=True)
                si2 = w2.ins.sync_info
                w2.ins.sync_info = sic(
                    on_wait=chunk,
                    on_update=si2.on_update if si2 is not None else [],
                )
        nc.sync.drain()
        nc.all_engine_barrier()
        popped = nc._tile_sem_poison_stack.pop()
        assert popped is self._sem_poison
        nc.clear_and_free_semaphores(list(self.sems.allocated().values()))
        nc.all_engine_barrier()


def r32(ap):
    return ap.bitcast(F32R)


MAX_WAITS = 1


def _split_excess_waits(nc):
    """This walrus build rejects instructions carrying more than one or two
    sync-wait commands (CTRL and pseudo-DMA structs especially). Rewrite every
    instruction with more than MAX_WAITS waits into a chain of same-engine
    NoOps each carrying MAX_WAITS waits, followed by the original."""
    import bass_rust

    n_new = 0
    for f in nc.m.functions:
        for bb in f.blocks:
            changed = False
            out = []
            for ins in bb.instructions:
                si = ins.sync_info
                waits = list(si.on_wait) if si is not None and si.on_wait else []
                if len(waits) > MAX_WAITS:
                    changed = True
                    ncar = len(waits) - MAX_WAITS
                    for i in range(0, ncar, MAX_WAITS):
                        chunk = waits[i : min(i + MAX_WAITS, ncar)]
                        nop = mybir.InstNoOp(
                            name=f"{ins.name}-wsplit{i}", ins=[], outs=[]
                        )
                        nop.engine = ins.engine
                        nop.sync_info = bass_rust.SyncInfo(
                            on_wait=chunk, on_update=[]
                        )
                        out.append(nop)
                        n_new += 1
                    ins.sync_info = bass_rust.SyncInfo(
                        on_wait=waits[ncar:], on_update=si.on_update
                    )
                out.append(ins)
            if changed:
                bb.instructions = out
    return n_new


def build_core_kernel(split_waits=True):
    nc = bass.Bass()
    xta = nc.declare_dram_parameter("xta", [DM_AUG, SEQ], F32, isOutput=False)
    wqkvt = nc.declare_dram_parameter("wqkvt", [DM_AUG, F_LOCAL], F32, isOutput=False)
    wot = nc.declare_dram_parameter("wot", [P, D_MODEL], F32, isOutput=False)
    wqwk = nc.declare_dram_parameter("wqwk", [D_HEAD, 2], F32, isOutput=False)
    out = nc.declare_dram_parameter("out", [SEQ, D_MODEL], F32, isOutput=True)

    xta_r = xta.rearrange("(c p) s -> p c s", p=P)       # [128, 9, 4096]
    wqkvt_r = wqkvt.rearrange("(c p) f -> p c f", p=P)   # [128, 9, 384]

    with ExitStack() as ctx:
        tc = ctx.enter_context(tile.TileContext(nc))

        const = ctx.enter_context(tc.tile_pool(name="const", bufs=1))
        persist = ctx.enter_context(tc.tile_pool(name="persist", bufs=1))

        ident = const.tile([P, P], F32)
        make_identity(nc, ident)
        # fp32 constant sources (memset cannot target fp32r, and fp32r matmul
        # operands must be produced by rounding compute ops)
        zeros_f32 = const.tile([P, QW], F32)
        nc.gpsimd.memset(zeros_f32[:], 0.0)
        ones_f32 = const.tile([P, D_HEAD], F32)
        nc.gpsimd.memset(ones_f32[:], 1.0)
        onesrow_st = const.tile([P, D_HEAD], F32)
        nc.gpsimd.memset(onesrow_st[:], 0.0)
        nc.gpsimd.memset(onesrow_st[0:1, :], 1.0)
        onesrow = const.tile([P, D_HEAD], F32R)
        nc.scalar.activation(onesrow[:], onesrow_st[:], AF.Copy)
        wqwk_sb = const.tile([D_HEAD, 2], F32)
        nc.sync.dma_start(wqwk_sb[:], wqwk[:])
        eps_t = const.tile([P, 1], F32)
        nc.gpsimd.memset(eps_t[:], EPS)
        wqkv_st = const.tile([P, DM_AUG // P, F_LOCAL], F32)
        nc.sync.dma_start(wqkv_st[:], wqkvt_r)
        wqkv_sb = const.tile([P, DM_AUG // P, F_LOCAL], F32R)
        nc.scalar.activation(wqkv_sb[:], wqkv_st[:], AF.Copy)
        wot_st = const.tile([P, D_MODEL], F32)
        nc.sync.dma_start(wot_st[:], wot[:])
        wot_sb = const.tile([P, D_MODEL], F32R)
        nc.scalar.activation(wot_sb[:], wot_st[:], AF.Copy)

        # attention operand tiles; rows >= 64 of the *hatT tiles stay zero so
        # every matmul contracts over a full 128 partitions
        qhatT = [persist.tile([P, SEQ], F32R, name=f"qhatT{h}", tag=f"qhatT{h}") for h in range(2)]
        khatT = [persist.tile([P, SEQ], F32R, name=f"khatT{h}", tag=f"khatT{h}") for h in range(2)]
        for t in (*qhatT, *khatT):
            for c in range(SEQ // QW):
                nc.scalar.activation(
                    t[D_HEAD:P, bass.ts(c, QW)], zeros_f32[0:D_HEAD, :], AF.Copy
                )
        vp = persist.tile([P, HEADS_LOCAL, KB, D_HEAD + 1], F32R)
        nc.scalar.activation(
            vp[:, :, :, D_HEAD : D_HEAD + 1], ones_f32[:, :, None], AF.Copy
        )
        z_nT = persist.tile([P, SEQ], F32R)
        recip_pad = persist.tile([P, QW], F32R)
        nc.scalar.activation(recip_pad[:], zeros_f32[:], AF.Copy)

        # ---------------- phase 1: QKV + RMSNorm + transposes ----------------
        with ExitStack() as p1:
            xpool = p1.enter_context(tc.tile_pool(name="xt", bufs=4))
            norm = p1.enter_context(tc.tile_pool(name="norm", bufs=3))
            qkps = p1.enter_context(tc.tile_pool(name="qkvps", bufs=3, space="PSUM"))
            tps = p1.enter_context(tc.tile_pool(name="tps", bufs=4, space="PSUM"))

            for sb in range(SB):
                ssl = bass.ts(sb, P)
                xt0 = xpool.tile([P, DM_AUG // P, P], F32)
                nc.sync.dma_start(xt0[:], xta_r[:, :, ssl])
                xt = xpool.tile([P, DM_AUG // P, P], F32R)
                nc.gpsimd.tensor_copy(xt[:], xt0[:])

                qkv_ps = qkps.tile([P, F_LOCAL], F32)
                for c in range(DM_AUG // P):
                    nc.tensor.matmul(
                        qkv_ps[:],
                        lhsT=xt[:, c, :],
                        rhs=wqkv_sb[:, c, :],
                        start=(c == 0),
                        stop=(c == DM_AUG // P - 1),
                    )

                # RMSNorm stats for the 4 (tensor, head) groups of 64
                sq = norm.tile([P, 4, D_HEAD], F32)
                qk_ps = qkv_ps[:, 0 : 4 * D_HEAD].rearrange(
                    "p (g d) -> p g d", g=4
                )
                nc.scalar.activation(sq[:], qk_ps, AF.Square)
                ss = norm.tile([P, 4], F32)
                nc.vector.tensor_reduce(
                    ss[:], sq[:], axis=mybir.AxisListType.X, op=ALU.add
                )
                rs = norm.tile([P, 4], F32)
                nc.scalar.activation(
                    rs[:], ss[:], AF.Sqrt, bias=eps_t[:], scale=1.0 / D_HEAD
                )
                rr = norm.tile([P, 4], F32)
                nc.vector.reciprocal(rr[:], rs[:])

                qk_hat = norm.tile([P, 4, D_HEAD], F32)
                nc.vector.tensor_tensor(
                    qk_hat[:],
                    qk_ps,
                    rr[:, :, None].to_broadcast((P, 4, D_HEAD)),
                    ALU.mult,
                )

                # V chunks for both heads
                nc.vector.tensor_copy(
                    vp[:, :, sb, 0:D_HEAD],
                    qkv_ps[:, 4 * D_HEAD : 6 * D_HEAD].rearrange(
                        "p (h d) -> p h d", h=2
                    ),
                )

                # transposes + wq/wk scaling into [d, s] layout
                for g, (dst, wcol) in enumerate(
                    [(qhatT[0], 0), (qhatT[1], 0), (khatT[0], 1), (khatT[1], 1)]
                ):
                    pt = tps.tile([D_HEAD, P], F32)
                    nc.tensor.transpose(pt[:], qk_hat[:, g, :], ident[:])
                    nc.vector.tensor_scalar_mul(
                        dst[0:D_HEAD, ssl], pt[:], wqwk_sb[:, wcol : wcol + 1]
                    )

        # ---------- phase 2+3: attention with inlined O-projection ----------
        # PSUM: score slots 2x3 banks (shared by O-proj tiles via tag),
        # z' accumulator 1 bank, broadcast 1 bank = 8.
        EXP_BATCH = 3
        with ExitStack() as p2:
            spool = p2.enter_context(tc.tile_pool(name="sps", bufs=2, space="PSUM"))
            zpool = p2.enter_context(tc.tile_pool(name="zps", bufs=1, space="PSUM"))
            bpool = p2.enter_context(tc.tile_pool(name="bps", bufs=1, space="PSUM"))
            ppool = p2.enter_context(tc.tile_pool(name="probs", bufs=4))
            osb = p2.enter_context(tc.tile_pool(name="osb", bufs=3))

            for qt in range(QT):
                qsl = bass.ts(qt, QW)
                for h in range(HEADS_LOCAL):
                    zps = zpool.tile([D_HEAD + 1, QW], F32, name="zps", tag="zps")
                    for kb0 in range(0, KB, EXP_BATCH):
                        nb = min(EXP_BATCH, KB - kb0)
                        sps = spool.tile(
                            [P, EXP_BATCH, QW], F32, name="sps", tag="sps"
                        )
                        for j in range(nb):
                            kb = kb0 + j
                            nc.tensor.matmul(
                                sps[:, j, :],
                                lhsT=khatT[h][:, bass.ts(kb, P)],
                                rhs=qhatT[h][:, qsl],
                                start=True,
                                stop=True,
                            )
                        probs = ppool.tile(
                            [P, EXP_BATCH, QW], F32R, name="probs", tag="probs"
                        )
                        nc.scalar.activation(
                            probs[:, 0:nb, :], sps[:, 0:nb, :], AF.Exp
                        )
                        for j in range(nb):
                            kb = kb0 + j
                            nc.tensor.matmul(
                                zps[:],
                                lhsT=vp[:, h, kb, :],
                                rhs=probs[:, j, :],
                                start=(kb == 0),
                                stop=(kb == KB - 1),
                                skip_group_check=True,
                            )
                    # normalize: z = z / rowsum (rowsum lives in zps row 64)
                    with nc.allow_low_precision(reason="fp32r operand rounding"):
                        nc.vector.reciprocal(
                            recip_pad[0:1, :], zps[D_HEAD : D_HEAD + 1, :]
                        )
                    bps = bpool.tile([D_HEAD, QW], F32, name="bps", tag="bps")
                    nc.tensor.matmul(
                        bps[:],
                        lhsT=onesrow[:],
                        rhs=recip_pad[:],
                        start=True,
                        stop=True,
                    )
                    rb = ppool.tile([D_HEAD, QW], F32, name="rb", tag="rb")
                    nc.vector.tensor_copy(rb[:], bps[:])
                    nc.vector.tensor_tensor(
                        z_nT[h * D_HEAD : (h + 1) * D_HEAD, qsl],
                        zps[0:D_HEAD, :],
                        rb[:],
                        ALU.mult,
                    )
                # O-projection for this q-tile's 4 s-blocks (both heads done);
                # ops tiles share the score-PSUM slots via the "sps" tag
                for sbl in range(QW // P):
                    sb = qt * (QW // P) + sbl
                    ops = spool.tile([P, D_MODEL], F32, name="ops", tag="sps")
                    for half in range(2):
                        nc.tensor.matmul(
                            ops[:, bass.ts(half, QW)],
                            lhsT=z_nT[:, bass.ts(sb, P)],
                            rhs=wot_sb[:, bass.ts(half, QW)],
                            start=True,
                            stop=True,
                        )
                    ot = osb.tile([P, D_MODEL], F32, name="ot", tag="ot")
                    nc.vector.tensor_copy(ot[:], ops[:])
                    nc.sync.dma_start(out[bass.ts(sb, P), :], ot[:])

    if split_waits:
        _split_excess_waits(nc)
    return nc


def shard_inputs(x, Wqkv, bqkv, Wo, bo, wq, wk):
    x2 = np.ascontiguousarray(np.asarray(x, dtype=np.float32).reshape(SEQ, D_MODEL))
    Wqkv = np.asarray(Wqkv, dtype=np.float32)
    bqkv = np.asarray(bqkv, dtype=np.float32)
    Wo = np.asarray(Wo, dtype=np.float32)
    wq = np.asarray(wq, dtype=np.float32)
    wk = np.asarray(wk, dtype=np.float32)

    xta = np.zeros((DM_AUG, SEQ), np.float32)
    xta[:D_MODEL] = x2.T
    xta[D_MODEL] = 1.0
    xta = np.ascontiguousarray(xta)

    wqwk = np.zeros((D_HEAD, 2), np.float32)
    wqwk[:, 0] = wq
    wqwk[:, 1] = wk

    in_maps = []
    for c in range(N_CORES):
        rows, brows = [], []
        for part in range(3):
            for h in (HEADS_LOCAL * c, HEADS_LOCAL * c + 1):
                sl = slice(part * D_MODEL + h * D_HEAD, part * D_MODEL + (h + 1) * D_HEAD)
                rows.append(Wqkv[sl])
                brows.append(bqkv[sl])
        Wl = np.concatenate(rows, 0)          # [384, 1024]
        bl = np.concatenate(brows, 0)         # [384]
        wqkvta = np.zeros((DM_AUG, F_LOCAL), np.float32)
        wqkvta[:D_MODEL] = Wl.T
        wqkvta[D_MODEL] = bl
        cols = slice(HEADS_LOCAL * c * D_HEAD, (HEADS_LOCAL * c + HEADS_LOCAL) * D_HEAD)
        wotc = np.ascontiguousarray(Wo[:, cols].T)    # [128, 1024]
        in_maps.append(
            {
                "xta": xta,
                "wqkvt": np.ascontiguousarray(wqkvta),
                "wot": wotc,
                "wqwk": wqwk,
            }
        )
    return in_maps


_NC_CACHE = {}
LAST_RESULT = None


def kernel(x, Wqkv, bqkv, Wo, bo, wq, wk):
    import os
    from concourse.bass_utils import run_bass_kernel_spmd

    global LAST_RESULT
    assert np.asarray(x).shape == (1, SEQ, D_MODEL)
    in_maps = shard_inputs(x, Wqkv, bqkv, Wo, bo, wq, wk)
    if "nc" not in _NC_CACHE:
        _NC_CACHE["nc"] = build_core_kernel()
    nc = _NC_CACHE["nc"]
    trace = bool(int(os.environ.get("BASS_KERNEL_TRACE", "0")))
    res = run_bass_kernel_spmd(nc, in_maps, list(range(N_CORES)), trace=trace)
    LAST_RESULT = res
    acc = np.zeros((SEQ, D_MODEL), np.float64)
    for c in range(N_CORES):
        acc += res.results[c]["out"].astype(np.float64)
    acc += np.asarray(bo, dtype=np.float64)
    return acc.astype(np.float32).reshape(1, SEQ, D_MODEL)



# revision 4
# speedup vs baseline: 1.2062x; 1.2062x over previous
"""Trainium2 Bass kernel for a 16-head attention block (d_model=1024, seq=4096).

Sharding: tensor-parallel over heads. Each of the 8 cores computes QKV
projections, RMSNorm(q,k), full softmax(QK^T)V attention for its 2 heads,
and a partial O-projection (its heads' slice of the contraction). The host
sums the 8 partial outputs (bf16 partials) and adds the output bias.

Per-core dataflow:
  phase 1: qkv[s,f] = xT_aug^T @ WqkvT_aug (bias via augmented ones row),
           RMSNorm over d_head on the [s_part, d_free] layout, PE-transpose
           q_hat/k_hat per head into [64, s] fp32r tiles (K=64 contraction,
           wq*wk folded into the k side), V -> bf16 [k, 65] chunks with a
           fused ones column for the softmax denominator.
  phase 2: per (q-tile 512, head): S[k,q] blocks via khatT.T @ qhatT (K=64),
           exp on ACT -> bf16 probs, then z[q,65] += probs_chunk.T @ V'
           accumulated over k-blocks with probs as the stationary operand
           (65-column moving operand halves PE time vs streaming probs).
           Row 64 = softmax denominator; normalize in q-major layout on DVE,
           PE-transpose both heads at once into z_nT[d_local, s] bf16.
  phase 3: out[s,dm] partial = z_nT.T @ WoT (bf16), PSUM->SBUF bf16 -> HBM.
"""

import numpy as np
from contextlib import ExitStack

import concourse.bass as bass
import concourse.tile as tile
from concourse import mybir
from concourse.masks import make_identity

F32 = mybir.dt.float32
F32R = mybir.dt.float32r
BF16 = mybir.dt.bfloat16
AF = mybir.ActivationFunctionType
ALU = mybir.AluOpType

D_MODEL = 1024
SEQ = 4096
N_HEADS = 16
D_HEAD = 64
N_CORES = 8
HEADS_LOCAL = 2
P = 128
F_LOCAL = 3 * HEADS_LOCAL * D_HEAD       # 384: [q0|q1|k0|k1|v0|v1]
DM_AUG = D_MODEL + P                     # 1152 rows: x^T plus ones-row block
NCH = DM_AUG // P                        # 9 contraction chunks
SB = SEQ // P                            # 32 s-blocks
QT = 8                                   # q-tiles of 512
QW = SEQ // QT                           # 512
QC = QW // P                             # 4 q-chunks of 128 per q-tile
KB = SEQ // P                            # 32 k-blocks
EXP_BATCH = 3
EPS = 1e-6


MAX_WAITS = 1


def _split_excess_waits(nc):
    """This walrus build rejects instructions carrying more than one or two
    sync-wait commands (CTRL and pseudo-DMA structs especially). Rewrite every
    instruction with more than MAX_WAITS waits into a chain of same-engine
    NoOps each carrying MAX_WAITS waits, followed by the original."""
    import bass_rust

    n_new = 0
    for f in nc.m.functions:
        for bb in f.blocks:
            changed = False
            out = []
            for ins in bb.instructions:
                si = ins.sync_info
                waits = list(si.on_wait) if si is not None and si.on_wait else []
                if len(waits) > MAX_WAITS:
                    changed = True
                    ncar = len(waits) - MAX_WAITS
                    for i in range(0, ncar, MAX_WAITS):
                        chunk = waits[i : min(i + MAX_WAITS, ncar)]
                        nop = mybir.InstNoOp(
                            name=f"{ins.name}-wsplit{i}", ins=[], outs=[]
                        )
                        nop.engine = ins.engine
                        nop.sync_info = bass_rust.SyncInfo(
                            on_wait=chunk, on_update=[]
                        )
                        out.append(nop)
                        n_new += 1
                    ins.sync_info = bass_rust.SyncInfo(
                        on_wait=waits[ncar:], on_update=si.on_update
                    )
                out.append(ins)
            if changed:
                bb.instructions = out
    return n_new


def build_core_kernel(split_waits=True):
    nc = bass.Bass()
    xta = nc.declare_dram_parameter("xta", [DM_AUG, SEQ], F32R, isOutput=False)
    wqkvt = nc.declare_dram_parameter("wqkvt", [DM_AUG, F_LOCAL], F32R, isOutput=False)
    wot = nc.declare_dram_parameter("wot", [P, D_MODEL], BF16, isOutput=False)
    wkc = nc.declare_dram_parameter("wkc", [D_HEAD, 1], F32, isOutput=False)
    out = nc.declare_dram_parameter("out", [SEQ, D_MODEL], BF16, isOutput=True)

    xta_r = xta.rearrange("(c p) s -> p c s", p=P)       # [128, 9, 4096]
    wqkvt_r = wqkvt.rearrange("(c p) f -> p c f", p=P)   # [128, 9, 384]

    with ExitStack() as ctx:
        tc = ctx.enter_context(tile.TileContext(nc))

        const = ctx.enter_context(tc.tile_pool(name="const", bufs=1))
        persist = ctx.enter_context(tc.tile_pool(name="persist", bufs=1))

        ident_f = const.tile([P, P], F32)
        make_identity(nc, ident_f)
        ident_r = const.tile([P, P], F32R)
        nc.scalar.activation(ident_r[:], ident_f[:], AF.Copy)
        ident_b = const.tile([P, P], BF16)
        nc.vector.tensor_copy(ident_b[:], ident_f[:])
        wkc_sb = const.tile([D_HEAD, 1], F32)
        nc.sync.dma_start(wkc_sb[:], wkc[:])
        eps_t = const.tile([P, 1], F32)
        nc.gpsimd.memset(eps_t[:], EPS)
        wqkv_sb = const.tile([P, NCH, F_LOCAL], F32R)
        nc.sync.dma_start(wqkv_sb[:], wqkvt_r)
        wot_sb = const.tile([P, D_MODEL], BF16)
        nc.sync.dma_start(wot_sb[:], wot[:])

        # attention operands: q_hat/k_hat in [64, s] fp32r (K=64 contraction),
        # V' in [k, 65] bf16 per (head, k-block) with ones denominator column
        qhatT = [persist.tile([D_HEAD, SEQ], F32R, name=f"qhatT{h}") for h in range(2)]
        khatT = [persist.tile([D_HEAD, SEQ], F32R, name=f"khatT{h}") for h in range(2)]
        vp = persist.tile([P, HEADS_LOCAL, KB, D_HEAD + 1], BF16)
        nc.gpsimd.memset(vp[:, :, :, D_HEAD : D_HEAD + 1], 1.0)
        z_nT = persist.tile([P, SEQ], BF16)

        # ---------------- phase 1: QKV + RMSNorm + transposes ----------------
        with ExitStack() as p1:
            xpool = p1.enter_context(tc.tile_pool(name="xt", bufs=4))
            norm = p1.enter_context(tc.tile_pool(name="norm", bufs=3))
            qkps = p1.enter_context(tc.tile_pool(name="qkvps", bufs=3, space="PSUM"))
            tps = p1.enter_context(tc.tile_pool(name="tps", bufs=4, space="PSUM"))

            for sb in range(SB):
                ssl = bass.ts(sb, P)
                xt = xpool.tile([P, NCH, P], F32R)
                nc.sync.dma_start(xt[:], xta_r[:, :, ssl])

                qkv_ps = qkps.tile([P, F_LOCAL], F32)
                for c in range(NCH):
                    nc.tensor.matmul(
                        qkv_ps[:],
                        lhsT=xt[:, c, :],
                        rhs=wqkv_sb[:, c, :],
                        start=(c == 0),
                        stop=(c == NCH - 1),
                    )

                # RMSNorm stats for the 4 (tensor, head) groups of 64
                qk_ps = qkv_ps[:, 0 : 4 * D_HEAD].rearrange(
                    "p (g d) -> p g d", g=4
                )
                sq = norm.tile([P, 4, D_HEAD], F32)
                nc.scalar.activation(sq[:], qk_ps, AF.Square)
                ss = norm.tile([P, 4], F32)
                nc.vector.tensor_reduce(
                    ss[:], sq[:], axis=mybir.AxisListType.X, op=ALU.add
                )
                rs = norm.tile([P, 4], F32)
                nc.scalar.activation(
                    rs[:], ss[:], AF.Sqrt, bias=eps_t[:], scale=1.0 / D_HEAD
                )
                rr = norm.tile([P, 4], F32)
                nc.vector.reciprocal(rr[:], rs[:])

                qk_hat = norm.tile([P, 4, D_HEAD], F32R)
                nc.vector.tensor_tensor(
                    qk_hat[:],
                    qk_ps,
                    rr[:, :, None].to_broadcast((P, 4, D_HEAD)),
                    ALU.mult,
                )

                # V chunks for both heads -> bf16 (ACT: same table as Square)
                nc.scalar.activation(
                    vp[:, :, sb, 0:D_HEAD],
                    qkv_ps[:, 4 * D_HEAD : 6 * D_HEAD].rearrange(
                        "p (h d) -> p h d", h=2
                    ),
                    AF.Copy,
                )

                # transposes into [d, s]; wq*wk folded into the k side
                for g, (dst, is_k) in enumerate(
                    [(qhatT[0], 0), (qhatT[1], 0), (khatT[0], 1), (khatT[1], 1)]
                ):
                    pt = tps.tile([D_HEAD, P], F32R)
                    nc.tensor.transpose(pt[:], qk_hat[:, g, :], ident_r[:])
                    if is_k:
                        nc.vector.tensor_scalar_mul(
                            dst[:, ssl], pt[:], wkc_sb[:]
                        )
                    else:
                        nc.scalar.activation(dst[:, ssl], pt[:], AF.Copy)

        # ---------- phase 2+3: attention with inlined O-projection ----------
        # PSUM banks: 2 score slots x3 banks (z-transpose tiles share via
        # tag), z accumulator 1 bank, O-proj 1 bank = 8.
        with ExitStack() as p2:
            spool = p2.enter_context(tc.tile_pool(name="sps", bufs=2, space="PSUM"))
            zqpool = p2.enter_context(tc.tile_pool(name="zqps", bufs=1, space="PSUM"))
            opool = p2.enter_context(tc.tile_pool(name="ops", bufs=1, space="PSUM"))
            ppool = p2.enter_context(tc.tile_pool(name="probs", bufs=3))
            znpool = p2.enter_context(tc.tile_pool(name="zn", bufs=2))
            rpool = p2.enter_context(tc.tile_pool(name="rcp", bufs=2))
            osb = p2.enter_context(tc.tile_pool(name="osb", bufs=3))

            def emit_oproj(qt):
                for sbl in range(QC):
                    sb = qt * QC + sbl
                    ot = osb.tile([P, D_MODEL], BF16, name="ot", tag="ot")
                    for half in range(2):
                        ops = opool.tile([P, QW], F32, name="ops", tag="ops")
                        nc.tensor.matmul(
                            ops[:],
                            lhsT=z_nT[:, bass.ts(sb, P)],
                            rhs=wot_sb[:, bass.ts(half, QW)],
                            start=True,
                            stop=True,
                        )
                        nc.vector.tensor_copy(ot[:, bass.ts(half, QW)], ops[:])
                    nc.sync.dma_start(out[bass.ts(sb, P), :], ot[:])

            for qt in range(QT):
                qsl = bass.ts(qt, QW)
                zn = znpool.tile([P, QC, P], BF16, name="zn", tag="zn")
                for h in range(HEADS_LOCAL):
                    zq = zqpool.tile([P, QC, D_HEAD + 1], F32, name="zq", tag="zq")
                    for kb0 in range(0, KB, EXP_BATCH):
                        nb = min(EXP_BATCH, KB - kb0)
                        sps = spool.tile(
                            [P, EXP_BATCH, QW], F32, name="sps", tag="sps"
                        )
                        for j in range(nb):
                            kb = kb0 + j
                            nc.tensor.matmul(
                                sps[:, j, :],
                                lhsT=khatT[h][:, bass.ts(kb, P)],
                                rhs=qhatT[h][:, qsl],
                                start=True,
                                stop=True,
                            )
                        probs = ppool.tile(
                            [P, EXP_BATCH, QW], BF16, name="probs", tag="probs"
                        )
                        nc.scalar.activation(
                            probs[:, 0:nb, :], sps[:, 0:nb, :], AF.Exp
                        )
                        # all 128 PV matmuls form ONE PSUM accumulation group
                        # (zq spans a single 2KB zero region): start marks the
                        # whole region pending-zero, each chunk's first touch
                        # overwrites, everything else accumulates
                        for j in range(nb):
                            kb = kb0 + j
                            for qc in range(QC):
                                nc.tensor.matmul(
                                    zq[:, qc, :],
                                    lhsT=probs[:, j, bass.ts(qc, P)],
                                    rhs=vp[:, h, kb, :],
                                    start=(kb == 0 and qc == 0),
                                    stop=(kb == KB - 1 and qc == QC - 1),
                                    skip_group_check=True,
                                )
                        # software-pipelined O-projection of the previous
                        # q-tile, emitted under the exp shadow
                        if kb0 == EXP_BATCH and h == 0 and qt > 0:
                            emit_oproj(qt - 1)
                    # normalize in q-major: z = z / rowsum (col 64)
                    rcp = rpool.tile([P, QC], F32, name="rcp", tag="rcp")
                    nc.vector.reciprocal(rcp[:], zq[:, :, D_HEAD])
                    nc.vector.tensor_tensor(
                        zn[:, :, bass.ts(h, D_HEAD)],
                        zq[:, :, 0:D_HEAD],
                        rcp[:, :, None].to_broadcast((P, QC, D_HEAD)),
                        ALU.mult,
                    )
                # transpose both heads at once into z_nT[d_local, s]
                ztp = spool.tile([P, QC, P], BF16, name="ztp", tag="sps")
                for qc in range(QC):
                    nc.tensor.transpose(ztp[:, qc, :], zn[:, qc, :], ident_b[:])
                    nc.vector.tensor_copy(
                        z_nT[:, qt * QW + qc * P : qt * QW + (qc + 1) * P],
                        ztp[:, qc, :],
                    )
            emit_oproj(QT - 1)

    if split_waits:
        _split_excess_waits(nc)
    return nc


def shard_inputs(x, Wqkv, bqkv, Wo, bo, wq, wk):
    import ml_dtypes

    x2 = np.ascontiguousarray(np.asarray(x, dtype=np.float32).reshape(SEQ, D_MODEL))
    Wqkv = np.asarray(Wqkv, dtype=np.float32)
    bqkv = np.asarray(bqkv, dtype=np.float32)
    Wo = np.asarray(Wo, dtype=np.float32)
    wq = np.asarray(wq, dtype=np.float32)
    wk = np.asarray(wk, dtype=np.float32)

    xta = np.zeros((DM_AUG, SEQ), np.float32)
    xta[:D_MODEL] = x2.T
    xta[D_MODEL] = 1.0
    xta = np.ascontiguousarray(xta)

    wkc = np.ascontiguousarray((wq * wk).reshape(D_HEAD, 1))

    in_maps = []
    for c in range(N_CORES):
        rows, brows = [], []
        for part in range(3):
            for h in (HEADS_LOCAL * c, HEADS_LOCAL * c + 1):
                sl = slice(part * D_MODEL + h * D_HEAD, part * D_MODEL + (h + 1) * D_HEAD)
                rows.append(Wqkv[sl])
                brows.append(bqkv[sl])
        Wl = np.concatenate(rows, 0)          # [384, 1024]
        bl = np.concatenate(brows, 0)         # [384]
        wqkvta = np.zeros((DM_AUG, F_LOCAL), np.float32)
        wqkvta[:D_MODEL] = Wl.T
        wqkvta[D_MODEL] = bl
        cols = slice(HEADS_LOCAL * c * D_HEAD, (HEADS_LOCAL * c + HEADS_LOCAL) * D_HEAD)
        wotc = np.ascontiguousarray(Wo[:, cols].T.astype(ml_dtypes.bfloat16))
        in_maps.append(
            {
                "xta": xta,
                "wqkvt": np.ascontiguousarray(wqkvta),
                "wot": wotc,
                "wkc": wkc,
            }
        )
    return in_maps


_NC_CACHE = {}
LAST_RESULT = None


def kernel(x, Wqkv, bqkv, Wo, bo, wq, wk):
    import os
    from concourse.bass_utils import run_bass_kernel_spmd

    global LAST_RESULT
    assert np.asarray(x).shape == (1, SEQ, D_MODEL)
    in_maps = shard_inputs(x, Wqkv, bqkv, Wo, bo, wq, wk)
    if "nc" not in _NC_CACHE:
        _NC_CACHE["nc"] = build_core_kernel()
    nc = _NC_CACHE["nc"]
    trace = bool(int(os.environ.get("BASS_KERNEL_TRACE", "0")))
    res = run_bass_kernel_spmd(nc, in_maps, list(range(N_CORES)), trace=trace)
    LAST_RESULT = res
    acc = np.zeros((SEQ, D_MODEL), np.float64)
    for c in range(N_CORES):
        acc += res.results[c]["out"].astype(np.float64)
    acc += np.asarray(bo, dtype=np.float64)
    return acc.astype(np.float32).reshape(1, SEQ, D_MODEL)


# revision 13
# speedup vs baseline: 1.3337x; 1.1057x over previous
"""Trainium2 Bass kernel for a 16-head attention block (d_model=1024, seq=4096).

Sharding: tensor-parallel over heads. Each of the 8 cores computes QKV
projections, RMSNorm(q,k), full softmax(QK^T)V attention for its 2 heads,
and a partial O-projection (its heads' slice of the contraction). The host
sums the 8 partial outputs (bf16 partials) and adds the output bias.

Per-core dataflow (k-first, attention is ACT/exp-bound so everything else
is arranged to hide under it):
  phase 1 (serial prefix, PE-bound): k,v projections only (fp32r, moving
           dim 256), RMSNorm(k) with wq*wk folded into the k side,
           PE-transpose k_hat into [64, s] fp32r tiles, V -> bf16 [k, 65]
           chunks with a fused ones column. The q side for the first
           q-tile is also produced here (bf16 GEMM, see below).
  phase 2 (ACT-bound steady state): per (q-tile 512, head):
           S[k,q] blocks via khatT.T @ qhatT (K=64 fp32r), exp on ACT ->
           bf16 probs, z[q,65] += probs_chunk.T @ V' with probs as the
           stationary operand (65-column moving operand halves PE time).
           Row 64 = softmax denominator; normalize in q-major on DVE,
           PE-transpose both heads at once into z_nT[d_local, s] bf16.
           The NEXT q-tile's q side runs under the exp shadow: bf16
           x @ Wq GEMM (N=128 bf16 runs at 1 cyc/row; fp32r would be 4x),
           RMSNorm(q) with a Newton-iteration rsqrt on DVE (keeps the
           ACT table on Exp), PE-transposes into qhatT.
  phase 3: out[s,dm] partial = z_nT.T @ WoT (bf16), PSUM->SBUF bf16 -> HBM,
           software-pipelined one q-tile behind attention.
"""

import numpy as np
from contextlib import ExitStack

import concourse.bass as bass
import concourse.tile as tile
from concourse import mybir
from concourse.masks import make_identity

F32 = mybir.dt.float32
F32R = mybir.dt.float32r
BF16 = mybir.dt.bfloat16
I32 = mybir.dt.int32
AF = mybir.ActivationFunctionType
ALU = mybir.AluOpType

D_MODEL = 1024
SEQ = 4096
N_HEADS = 16
D_HEAD = 64
N_CORES = 8
HEADS_LOCAL = 2
P = 128
DM_AUG = D_MODEL + P                     # 1152 rows: x^T plus ones-row block
NCH = DM_AUG // P                        # 9 contraction chunks
KV_LOCAL = 2 * HEADS_LOCAL * D_HEAD      # 256: [k0|k1|v0|v1]
Q_LOCAL = HEADS_LOCAL * D_HEAD           # 128: [q0|q1]
SB = SEQ // P                            # 32 s-blocks
QT = 8                                   # q-tiles of 512
QW = SEQ // QT                           # 512
QC = QW // P                             # 4 q-chunks of 128 per q-tile
KB = SEQ // P                            # 32 k-blocks
EXP_BATCH = 3
EPS = 1e-6
RSQRT_MAGIC = 0x5F3759DF


MAX_WAITS = 1


def _split_excess_waits(nc):
    """This walrus build rejects instructions carrying more than one or two
    sync-wait commands (CTRL and pseudo-DMA structs especially). Rewrite every
    instruction with more than MAX_WAITS waits into a chain of same-engine
    NoOps each carrying MAX_WAITS waits, followed by the original."""
    import bass_rust

    n_new = 0
    for f in nc.m.functions:
        for bb in f.blocks:
            changed = False
            out = []
            for ins in bb.instructions:
                si = ins.sync_info
                waits = list(si.on_wait) if si is not None and si.on_wait else []
                if len(waits) > MAX_WAITS:
                    changed = True
                    ncar = len(waits) - MAX_WAITS
                    for i in range(0, ncar, MAX_WAITS):
                        chunk = waits[i : min(i + MAX_WAITS, ncar)]
                        nop = mybir.InstNoOp(
                            name=f"{ins.name}-wsplit{i}", ins=[], outs=[]
                        )
                        nop.engine = ins.engine
                        nop.sync_info = bass_rust.SyncInfo(
                            on_wait=chunk, on_update=[]
                        )
                        out.append(nop)
                        n_new += 1
                    ins.sync_info = bass_rust.SyncInfo(
                        on_wait=waits[ncar:], on_update=si.on_update
                    )
                out.append(ins)
            if changed:
                bb.instructions = out
    return n_new


def build_core_kernel(split_waits=True):
    nc = bass.Bass()
    xtb = nc.declare_dram_parameter("xtb", [DM_AUG, SEQ], BF16, isOutput=False)
    wkvt = nc.declare_dram_parameter("wkvt", [DM_AUG, KV_LOCAL], BF16, isOutput=False)
    wqt = nc.declare_dram_parameter("wqt", [DM_AUG, Q_LOCAL], BF16, isOutput=False)
    wot = nc.declare_dram_parameter("wot", [P, D_MODEL], BF16, isOutput=False)
    wkc = nc.declare_dram_parameter("wkc", [D_HEAD, 1], F32, isOutput=False)
    out = nc.declare_dram_parameter("out", [SEQ, D_MODEL], BF16, isOutput=True)

    xtb_r = xtb.rearrange("(c p) s -> p c s", p=P)       # [128, 9, 4096]
    wkvt_r = wkvt.rearrange("(c p) f -> p c f", p=P)     # [128, 9, 256]
    wqt_r = wqt.rearrange("(c p) f -> p c f", p=P)       # [128, 9, 128]

    with ExitStack() as ctx:
        tc = ctx.enter_context(tile.TileContext(nc))

        const = ctx.enter_context(tc.tile_pool(name="const", bufs=1))
        persist = ctx.enter_context(tc.tile_pool(name="persist", bufs=1))

        ident_f = const.tile([P, P], F32)
        make_identity(nc, ident_f)
        ident_r = const.tile([P, P], F32R)
        nc.scalar.activation(ident_r[:], ident_f[:], AF.Copy)
        ident_b = const.tile([P, P], BF16)
        nc.vector.tensor_copy(ident_b[:], ident_f[:])
        wkc_sb = const.tile([D_HEAD, 1], F32)
        nc.sync.dma_start(wkc_sb[:], wkc[:])
        eps_t = const.tile([P, 1], F32)
        nc.gpsimd.memset(eps_t[:], EPS)
        wkv_sb = const.tile([P, NCH, KV_LOCAL], BF16)
        nc.sync.dma_start(wkv_sb[:], wkvt_r)
        wq_sb = const.tile([P, NCH, Q_LOCAL], BF16)
        nc.sync.dma_start(wq_sb[:], wqt_r)
        wot_sb = const.tile([P, D_MODEL], BF16)
        nc.sync.dma_start(wot_sb[:], wot[:])

        # attention operands: q_hat/k_hat in [64, s] fp32r (K=64 contraction),
        # V' in [k, 65] bf16 per (head, k-block) with ones denominator column
        qhatT = [persist.tile([D_HEAD, SEQ], F32R, name=f"qhatT{h}") for h in range(2)]
        khatT = [persist.tile([D_HEAD, SEQ], F32R, name=f"khatT{h}") for h in range(2)]
        vp = persist.tile([P, HEADS_LOCAL, KB, D_HEAD + 1], BF16)
        nc.gpsimd.memset(vp[:, :, :, D_HEAD : D_HEAD + 1], 1.0)
        z_nT = persist.tile([P, SEQ], BF16)

        # resident bf16 copy of x^T (augmented): 8 chunk tiles of 512 s-cols,
        # loaded once (1KB descriptor runs; serves both kv- and q-GEMMs)
        xball = [
            persist.tile([P, NCH, QW], BF16, name=f"xb{d}") for d in range(QT)
        ]
        for d in range(QT):
            nc.sync.dma_start(xball[d][:], xtb_r[:, :, bass.ts(d, QW)])

        qnorm = ctx.enter_context(tc.tile_pool(name="qnorm", bufs=2))

        def emit_q_side_sb(sb, qpool, tpool, rsqrt_act=False):
            """bf16 x @ Wq for one s-block + RMSNorm(q) + PE transposes into
            qhatT. rsqrt_act: use ACT Rsqrt (phase-1 table) instead of the
            DVE Newton rsqrt (phase 2 keeps the ACT table on Exp)."""
            if True:
                ssl = bass.ts(sb, P)
                xbt = xball[sb // QC]
                xsl = bass.ts(sb % QC, P)
                qps = qpool.tile([P, Q_LOCAL], F32, name="qps", tag=qpool.name_tag)
                for c in range(NCH):
                    nc.tensor.matmul(
                        qps[:],
                        lhsT=xbt[:, c, xsl],
                        rhs=wq_sb[:, c, :],
                        start=(c == 0),
                        stop=(c == NCH - 1),
                    )
                qsb = qnorm.tile([P, Q_LOCAL], F32, name="qsb", tag="qsb")
                nc.vector.tensor_copy(qsb[:], qps[:])
                qg = qsb[:].rearrange("p (g d) -> p g d", g=2)
                sq = qnorm.tile([P, 2, D_HEAD], F32, name="qsq", tag="qsq")
                nc.vector.tensor_tensor(sq[:], qg, qg, ALU.mult)
                ss = qnorm.tile([P, 2], F32, name="qss", tag="qss")
                nc.vector.tensor_reduce(
                    ss[:], sq[:], axis=mybir.AxisListType.X, op=ALU.add
                )
                # rr = rsqrt(ss/64 + eps) via bit-trick seed + 2 Newton steps
                if rsqrt_act:
                    yrs = qnorm.tile([P, 2], F32, name="qrs", tag="qrs")
                    nc.scalar.activation(
                        yrs[:], ss[:], AF.Sqrt, bias=eps_t[:], scale=1.0 / D_HEAD
                    )
                    yact = qnorm.tile([P, 2], F32, name="qra", tag="qra")
                    nc.vector.reciprocal(yact[:], yrs[:])
                    y = yact[:]
                else:
                    ms = qnorm.tile([P, 2], F32, name="qms", tag="qms")
                    nc.vector.tensor_scalar(
                        ms[:], ss[:], 1.0 / D_HEAD, EPS, op0=ALU.mult, op1=ALU.add
                    )
                    xh = qnorm.tile([P, 2], F32, name="qxh", tag="qxh")
                    nc.vector.tensor_scalar(xh[:], ms[:], 0.5, None, op0=ALU.mult)
                    iy = qnorm.tile([P, 2], I32, name="qiy", tag="qiy")
                    nc.vector.tensor_scalar(
                        iy[:], ms[:].bitcast(I32), 1, None, op0=ALU.logical_shift_right
                    )
                    nc.vector.tensor_scalar(
                        iy[:], iy[:], -1, RSQRT_MAGIC, op0=ALU.mult, op1=ALU.add
                    )
                    y = iy[:].bitcast(F32)
                    for it in range(2):
                        y2 = qnorm.tile([P, 2], F32, name=f"qy2_{it}", tag=f"qy2_{it}")
                        nc.vector.tensor_tensor(y2[:], y, y, ALU.mult)
                        nc.vector.tensor_tensor(y2[:], y2[:], xh[:], ALU.mult)
                        nc.vector.tensor_scalar(
                            y2[:], y2[:], -1.0, 1.5, op0=ALU.mult, op1=ALU.add
                        )
                        yn = qnorm.tile([P, 2], F32, name=f"qyn_{it}", tag=f"qyn_{it}")
                        nc.vector.tensor_tensor(yn[:], y, y2[:], ALU.mult)
                        y = yn[:]
                q_hat = qnorm.tile([P, 2, D_HEAD], F32R, name="qhat", tag="qhat")
                nc.vector.tensor_tensor(
                    q_hat[:], qg, y[:, :, None].to_broadcast((P, 2, D_HEAD)), ALU.mult
                )
                for h in range(2):
                    pt = tpool.tile(
                        [D_HEAD, P], F32R, name="qpt", tag=tpool.name_tag
                    )
                    nc.tensor.transpose(pt[:], q_hat[:, h, :], ident_r[:])
                    nc.vector.tensor_copy(qhatT[h][:, ssl], pt[:])

        # ------------- phase 1: K/V projections + RMSNorm(k) + V' -------------
        with ExitStack() as p1:
            norm = p1.enter_context(tc.tile_pool(name="norm", bufs=6))
            qkps = p1.enter_context(tc.tile_pool(name="kvps", bufs=4, space="PSUM"))
            tps = p1.enter_context(tc.tile_pool(name="tps", bufs=4, space="PSUM"))

            class _P1Pool:
                name_tag = "kvps"

                @staticmethod
                def tile(shape, dt, name=None, tag=None):
                    return qkps.tile(shape, dt, name=name, tag="kvps")

            class _P1TPool:
                name_tag = "tps"

                @staticmethod
                def tile(shape, dt, name=None, tag=None):
                    return tps.tile(shape, dt, name=name, tag="tps")

            for sb in range(SB):
                ssl = bass.ts(sb, P)
                xbt = xball[sb // QC]
                xsl = bass.ts(sb % QC, P)

                kv_ps = qkps.tile([P, KV_LOCAL], F32, name="kv_ps", tag="kvps")
                for c in range(NCH):
                    nc.tensor.matmul(
                        kv_ps[:],
                        lhsT=xbt[:, c, xsl],
                        rhs=wkv_sb[:, c, :],
                        start=(c == 0),
                        stop=(c == NCH - 1),
                    )

                # RMSNorm stats for the 2 k heads
                k_ps = kv_ps[:, 0 : 2 * D_HEAD].rearrange("p (g d) -> p g d", g=2)
                sq = norm.tile([P, 2, D_HEAD], F32)
                nc.scalar.activation(sq[:], k_ps, AF.Square)
                ss = norm.tile([P, 2], F32)
                nc.vector.tensor_reduce(
                    ss[:], sq[:], axis=mybir.AxisListType.X, op=ALU.add
                )
                rs = norm.tile([P, 2], F32)
                nc.scalar.activation(
                    rs[:], ss[:], AF.Sqrt, bias=eps_t[:], scale=1.0 / D_HEAD
                )
                rr = norm.tile([P, 2], F32)
                nc.vector.reciprocal(rr[:], rs[:])

                k_hat = norm.tile([P, 2, D_HEAD], F32R)
                nc.vector.tensor_tensor(
                    k_hat[:],
                    k_ps,
                    rr[:, :, None].to_broadcast((P, 2, D_HEAD)),
                    ALU.mult,
                )

                # V chunks for both heads -> bf16 (ACT: same table as Square)
                nc.scalar.activation(
                    vp[:, :, sb, 0:D_HEAD],
                    kv_ps[:, 2 * D_HEAD : 4 * D_HEAD].rearrange(
                        "p (h d) -> p h d", h=2
                    ),
                    AF.Copy,
                )

                # transposes into [d, s]; wq*wk folded into the k side
                for h in range(2):
                    pt = tps.tile([D_HEAD, P], F32R, name="pt", tag="tps")
                    nc.tensor.transpose(pt[:], k_hat[:, h, :], ident_r[:])
                    nc.vector.tensor_scalar_mul(khatT[h][:, ssl], pt[:], wkc_sb[:])

                # q side of the first q-tile, interleaved into the prefix
                # tail (ACT Rsqrt: phase 1 owns the rsqrt table)
                if sb >= SB - 2 * QC and (sb - SB) % 2 == 0:
                    emit_q_side_sb((sb - (SB - 2 * QC)) // 2, _P1Pool, _P1TPool,
                                   rsqrt_act=True)



        # ---------- phase 2+3: attention with inlined O-projection ----------
        # PSUM banks: 2 score slots x3 banks, z accumulator 1 bank, shared
        # utility bank ("ops": O-proj / next-q-tile GEMM+transposes / ztp) = 8
        with ExitStack() as p2:
            spool = p2.enter_context(tc.tile_pool(name="sps", bufs=2, space="PSUM"))
            zqpool = p2.enter_context(tc.tile_pool(name="zqps", bufs=1, space="PSUM"))
            opool = p2.enter_context(tc.tile_pool(name="ops", bufs=1, space="PSUM"))
            ppool = p2.enter_context(tc.tile_pool(name="probs", bufs=3))
            znpool = p2.enter_context(tc.tile_pool(name="zn", bufs=2))
            rpool = p2.enter_context(tc.tile_pool(name="rcp", bufs=2))
            osb = p2.enter_context(tc.tile_pool(name="osb", bufs=3))

            class _P2Pool:
                name_tag = "ops"

                @staticmethod
                def tile(shape, dt, name=None, tag=None):
                    return opool.tile(shape, dt, name=name, tag="ops")

            def emit_oproj(qt, final=False):
                # final q-tile: S slots are free, so pipeline the matmuls
                # 2-wide through them and put half the copies on the idle ACT
                for sbl in range(QC):
                    sb = qt * QC + sbl
                    ot = osb.tile([P, D_MODEL], BF16, name="ot", tag="ot")
                    for half in range(2):
                        if final:
                            ops = spool.tile([P, QW], F32, name="ops", tag="sps")
                        else:
                            ops = opool.tile([P, QW], F32, name="ops", tag="ops")
                        nc.tensor.matmul(
                            ops[:],
                            lhsT=z_nT[:, bass.ts(sb, P)],
                            rhs=wot_sb[:, bass.ts(half, QW)],
                            start=True,
                            stop=True,
                        )
                        if final and half == 0:
                            nc.scalar.activation(
                                ot[:, bass.ts(half, QW)], ops[:], AF.Copy
                            )
                        else:
                            nc.vector.tensor_copy(ot[:, bass.ts(half, QW)], ops[:])
                    nc.sync.dma_start(out[bass.ts(sb, P), :], ot[:])

            for qt in range(QT):
                qsl = bass.ts(qt, QW)
                zn = znpool.tile([P, QC, P], BF16, name="zn", tag="zn")
                for h in range(HEADS_LOCAL):
                    zq = zqpool.tile([P, QC, D_HEAD + 1], F32, name="zq", tag="zq")
                    for kb0 in range(0, KB, EXP_BATCH):
                        nb = min(EXP_BATCH, KB - kb0)
                        sps = spool.tile(
                            [P, EXP_BATCH, QW], F32, name="sps", tag="sps"
                        )
                        for j in range(nb):
                            kb = kb0 + j
                            nc.tensor.matmul(
                                sps[:, j, :],
                                lhsT=khatT[h][:, bass.ts(kb, P)],
                                rhs=qhatT[h][:, qsl],
                                start=True,
                                stop=True,
                            )
                        probs = ppool.tile(
                            [P, EXP_BATCH, QW], BF16, name="probs", tag="probs"
                        )
                        nc.scalar.activation(
                            probs[:, 0:nb, :], sps[:, 0:nb, :], AF.Exp
                        )
                        # all 128 PV matmuls form ONE PSUM accumulation group
                        # (zq spans a single 2KB zero region): start marks the
                        # whole region pending-zero, each chunk's first touch
                        # overwrites, everything else accumulates
                        for j in range(nb):
                            kb = kb0 + j
                            for qc in range(QC):
                                nc.tensor.matmul(
                                    zq[:, qc, :],
                                    lhsT=probs[:, j, bass.ts(qc, P)],
                                    rhs=vp[:, h, kb, :],
                                    start=(kb == 0 and qc == 0),
                                    stop=(kb == KB - 1 and qc == QC - 1),
                                    skip_group_check=True,
                                )
                        # software-pipelined work emitted under the exp shadow:
                        # h0: O-projection of the previous q-tile
                        # h1: q side (GEMM+norm+transposes) of the next q-tile
                        if kb0 == EXP_BATCH and h == 0 and qt > 0:
                            emit_oproj(qt - 1)
                        if kb0 == EXP_BATCH and h == 1 and qt < QT - 1:
                            for sbl in range(QC):
                                emit_q_side_sb((qt + 1) * QC + sbl, _P2Pool, _P2Pool)
                    # normalize in q-major: z = z / rowsum (col 64)
                    rcp = rpool.tile([P, QC], F32, name="rcp", tag="rcp")
                    nc.vector.reciprocal(rcp[:], zq[:, :, D_HEAD])
                    nc.vector.tensor_tensor(
                        zn[:, :, bass.ts(h, D_HEAD)],
                        zq[:, :, 0:D_HEAD],
                        rcp[:, :, None].to_broadcast((P, QC, D_HEAD)),
                        ALU.mult,
                    )
                # transpose both heads at once into z_nT[d_local, s]
                ztp = opool.tile([P, QC, P], BF16, name="ztp", tag="ops")
                for qc in range(QC):
                    nc.tensor.transpose(ztp[:, qc, :], zn[:, qc, :], ident_b[:])
                    nc.vector.tensor_copy(
                        z_nT[:, qt * QW + qc * P : qt * QW + (qc + 1) * P],
                        ztp[:, qc, :],
                    )
            emit_oproj(QT - 1, final=True)

    if split_waits:
        _split_excess_waits(nc)
    return nc


def shard_inputs(x, Wqkv, bqkv, Wo, bo, wq, wk):
    import ml_dtypes

    x2 = np.ascontiguousarray(np.asarray(x, dtype=np.float32).reshape(SEQ, D_MODEL))
    Wqkv = np.asarray(Wqkv, dtype=np.float32)
    bqkv = np.asarray(bqkv, dtype=np.float32)
    Wo = np.asarray(Wo, dtype=np.float32)
    wq = np.asarray(wq, dtype=np.float32)
    wk = np.asarray(wk, dtype=np.float32)

    xta = np.zeros((DM_AUG, SEQ), np.float32)
    xta[:D_MODEL] = x2.T
    xta[D_MODEL] = 1.0
    xtb = np.ascontiguousarray(xta.astype(ml_dtypes.bfloat16))

    wkc = np.ascontiguousarray((wq * wk).reshape(D_HEAD, 1))

    in_maps = []
    for c in range(N_CORES):
        rows, brows = [], []
        for part in range(3):
            for h in (HEADS_LOCAL * c, HEADS_LOCAL * c + 1):
                sl = slice(part * D_MODEL + h * D_HEAD, part * D_MODEL + (h + 1) * D_HEAD)
                rows.append(Wqkv[sl])
                brows.append(bqkv[sl])
        Wl = np.concatenate(rows, 0)          # [384, 1024] rows [q0|q1|k0|k1|v0|v1]
        bl = np.concatenate(brows, 0)         # [384]
        wqkvta = np.zeros((DM_AUG, 384), np.float32)
        wqkvta[:D_MODEL] = Wl.T
        wqkvta[D_MODEL] = bl
        wkvt = np.ascontiguousarray(
            wqkvta[:, Q_LOCAL:].astype(ml_dtypes.bfloat16)
        )                                                              # [1152, 256]
        wqt = np.ascontiguousarray(
            wqkvta[:, :Q_LOCAL].astype(ml_dtypes.bfloat16)
        )                                                              # [1152, 128]
        cols = slice(HEADS_LOCAL * c * D_HEAD, (HEADS_LOCAL * c + HEADS_LOCAL) * D_HEAD)
        wotc = np.ascontiguousarray(Wo[:, cols].T.astype(ml_dtypes.bfloat16))
        in_maps.append(
            {
                "xtb": xtb,
                "wkvt": wkvt,
                "wqt": wqt,
                "wot": wotc,
                "wkc": wkc,
            }
        )
    return in_maps


_NC_CACHE = {}
LAST_RESULT = None


def kernel(x, Wqkv, bqkv, Wo, bo, wq, wk):
    import os
    from concourse.bass_utils import run_bass_kernel_spmd

    global LAST_RESULT
    assert np.asarray(x).shape == (1, SEQ, D_MODEL)
    in_maps = shard_inputs(x, Wqkv, bqkv, Wo, bo, wq, wk)
    if "nc" not in _NC_CACHE:
        _NC_CACHE["nc"] = build_core_kernel()
    nc = _NC_CACHE["nc"]
    trace = bool(int(os.environ.get("BASS_KERNEL_TRACE", "0")))
    res = run_bass_kernel_spmd(nc, in_maps, list(range(N_CORES)), trace=trace)
    LAST_RESULT = res
    acc = np.zeros((SEQ, D_MODEL), np.float64)
    for c in range(N_CORES):
        acc += res.results[c]["out"].astype(np.float64)
    acc += np.asarray(bo, dtype=np.float64)
    return acc.astype(np.float32).reshape(1, SEQ, D_MODEL)


# revision 18
# speedup vs baseline: 1.3565x; 1.0171x over previous
"""Trainium2 Bass kernel for a 16-head attention block (d_model=1024, seq=4096).

Sharding: tensor-parallel over heads. Each of the 8 cores computes QKV
projections, RMSNorm(q,k), full softmax(QK^T)V attention for its 2 heads,
and a partial O-projection (its heads' slice of the contraction). The host
sums the 8 partial outputs (bf16 partials) and adds the output bias.

Per-core dataflow (k-first, attention is ACT/exp-bound so everything else
is arranged to hide under it):
  phase 1 (serial prefix, PE-bound): k,v projections only (fp32r, moving
           dim 256), RMSNorm(k) with wq*wk folded into the k side,
           PE-transpose k_hat into [64, s] fp32r tiles, V -> bf16 [k, 65]
           chunks with a fused ones column. The q side for the first
           q-tile is also produced here (bf16 GEMM, see below).
  phase 2 (ACT-bound steady state): per (q-tile 512, head):
           S[k,q] blocks via khatT.T @ qhatT (K=64 fp32r), exp on ACT ->
           bf16 probs, z[q,65] += probs_chunk.T @ V' with probs as the
           stationary operand (65-column moving operand halves PE time).
           Row 64 = softmax denominator; normalize in q-major on DVE,
           PE-transpose both heads at once into z_nT[d_local, s] bf16.
           The NEXT q-tile's q side runs under the exp shadow: bf16
           x @ Wq GEMM (N=128 bf16 runs at 1 cyc/row; fp32r would be 4x),
           RMSNorm(q) with a Newton-iteration rsqrt on DVE (keeps the
           ACT table on Exp), PE-transposes into qhatT.
  phase 3: out[s,dm] partial = z_nT.T @ WoT (bf16), PSUM->SBUF bf16 -> HBM,
           software-pipelined one q-tile behind attention.
"""

import numpy as np
from contextlib import ExitStack

import concourse.bass as bass
import concourse.tile as tile
from concourse import mybir
from concourse.masks import make_identity

F32 = mybir.dt.float32
F32R = mybir.dt.float32r
BF16 = mybir.dt.bfloat16
I32 = mybir.dt.int32
AF = mybir.ActivationFunctionType
ALU = mybir.AluOpType

D_MODEL = 1024
SEQ = 4096
N_HEADS = 16
D_HEAD = 64
N_CORES = 8
HEADS_LOCAL = 2
P = 128
DM_AUG = D_MODEL + P                     # 1152 rows: x^T plus ones-row block
NCH = DM_AUG // P                        # 9 contraction chunks
KV_LOCAL = 2 * HEADS_LOCAL * D_HEAD      # 256: [k0|k1|v0|v1]
Q_LOCAL = HEADS_LOCAL * D_HEAD           # 128: [q0|q1]
SB = SEQ // P                            # 32 s-blocks
QT = 8                                   # q-tiles of 512
QW = SEQ // QT                           # 512
QC = QW // P                             # 4 q-chunks of 128 per q-tile
KB = SEQ // P                            # 32 k-blocks
EXP_BATCH = 3
EPS = 1e-6
RSQRT_MAGIC = 0x5F3759DF


MAX_WAITS = 1


def _split_excess_waits(nc):
    """This walrus build rejects instructions carrying more than one or two
    sync-wait commands (CTRL and pseudo-DMA structs especially). Rewrite every
    instruction with more than MAX_WAITS waits into a chain of same-engine
    NoOps each carrying MAX_WAITS waits, followed by the original."""
    import bass_rust

    n_new = 0
    for f in nc.m.functions:
        for bb in f.blocks:
            changed = False
            out = []
            for ins in bb.instructions:
                si = ins.sync_info
                waits = list(si.on_wait) if si is not None and si.on_wait else []
                if len(waits) > MAX_WAITS:
                    changed = True
                    ncar = len(waits) - MAX_WAITS
                    for i in range(0, ncar, MAX_WAITS):
                        chunk = waits[i : min(i + MAX_WAITS, ncar)]
                        nop = mybir.InstNoOp(
                            name=f"{ins.name}-wsplit{i}", ins=[], outs=[]
                        )
                        nop.engine = ins.engine
                        nop.sync_info = bass_rust.SyncInfo(
                            on_wait=chunk, on_update=[]
                        )
                        out.append(nop)
                        n_new += 1
                    ins.sync_info = bass_rust.SyncInfo(
                        on_wait=waits[ncar:], on_update=si.on_update
                    )
                out.append(ins)
            if changed:
                bb.instructions = out
    return n_new


def build_core_kernel(split_waits=True):
    nc = bass.Bass()
    xtb = nc.declare_dram_parameter("xtb", [DM_AUG, SEQ], BF16, isOutput=False)
    wkvt = nc.declare_dram_parameter("wkvt", [DM_AUG, KV_LOCAL], BF16, isOutput=False)
    wqt = nc.declare_dram_parameter("wqt", [DM_AUG, Q_LOCAL], BF16, isOutput=False)
    wot = nc.declare_dram_parameter("wot", [P, D_MODEL], BF16, isOutput=False)
    wkc = nc.declare_dram_parameter("wkc", [D_HEAD, 1], F32, isOutput=False)
    out = nc.declare_dram_parameter("out", [SEQ, D_MODEL], BF16, isOutput=True)

    xtb_r = xtb.rearrange("(c p) s -> p c s", p=P)       # [128, 9, 4096]
    wkvt_r = wkvt.rearrange("(c p) f -> p c f", p=P)     # [128, 9, 256]
    wqt_r = wqt.rearrange("(c p) f -> p c f", p=P)       # [128, 9, 128]

    with ExitStack() as ctx:
        tc = ctx.enter_context(tile.TileContext(nc))

        const = ctx.enter_context(tc.tile_pool(name="const", bufs=1))
        persist = ctx.enter_context(tc.tile_pool(name="persist", bufs=1))

        # DMA order matters: the whole prefix waits on x chunk 0 + Wkv
        xb0 = const.tile([P, NCH, QW], BF16, name="xb0")
        nc.sync.dma_start(xb0[:, :, 0:QW // 2], xtb_r[:, :, 0 : QW // 2])
        wkv_sb = const.tile([P, NCH, KV_LOCAL], BF16)
        nc.sync.dma_start(wkv_sb[:], wkvt_r)
        nc.sync.dma_start(xb0[:, :, QW // 2 : QW], xtb_r[:, :, QW // 2 : QW])
        ident_f = const.tile([P, P], F32)
        make_identity(nc, ident_f)
        ident_r = const.tile([P, P], F32R)
        nc.scalar.activation(ident_r[:], ident_f[:], AF.Copy)
        ident_b = const.tile([P, P], BF16)
        nc.vector.tensor_copy(ident_b[:], ident_f[:])
        wkc_sb = const.tile([D_HEAD, 1], F32)
        nc.sync.dma_start(wkc_sb[:], wkc[:])
        eps_t = const.tile([P, 1], F32)
        nc.gpsimd.memset(eps_t[:], EPS)
        wq_sb = const.tile([P, NCH, Q_LOCAL], BF16)
        nc.sync.dma_start(wq_sb[:], wqt_r)
        wot_sb = const.tile([P, D_MODEL], BF16)
        nc.sync.dma_start(wot_sb[:], wot[:])

        # attention operands: q_hat/k_hat in [64, s] fp32r (K=64 contraction),
        # V' in [k, 65] bf16 per (head, k-block) with ones denominator column
        qhatT = [persist.tile([D_HEAD, SEQ], F32R, name=f"qhatT{h}") for h in range(2)]
        khatT = [persist.tile([D_HEAD, SEQ], F32R, name=f"khatT{h}") for h in range(2)]
        vp = persist.tile([P, HEADS_LOCAL, KB, D_HEAD + 1], BF16)
        nc.gpsimd.memset(vp[:, :, :, D_HEAD : D_HEAD + 1], 1.0)
        z_nT = persist.tile([P, SEQ], BF16)

        # resident bf16 copy of x^T (augmented): 8 chunk tiles of 512 s-cols,
        # loaded once (1KB descriptor runs; serves both kv- and q-GEMMs)
        xball = [xb0] + [
            persist.tile([P, NCH, QW], BF16, name=f"xb{d}") for d in range(1, QT)
        ]
        for d in range(1, QT):
            nc.sync.dma_start(xball[d][:], xtb_r[:, :, bass.ts(d, QW)])

        qnorm = ctx.enter_context(tc.tile_pool(name="qnorm", bufs=2))

        def emit_q_side_sb(sb, qpool, tpool, rsqrt_act=False):
            """bf16 x @ Wq for one s-block + RMSNorm(q) + PE transposes into
            qhatT. rsqrt_act: use ACT Rsqrt (phase-1 table) instead of the
            DVE Newton rsqrt (phase 2 keeps the ACT table on Exp)."""
            if True:
                ssl = bass.ts(sb, P)
                xbt = xball[sb // QC]
                xsl = bass.ts(sb % QC, P)
                qps = qpool.tile([P, Q_LOCAL], F32, name="qps", tag=qpool.name_tag)
                for c in range(NCH):
                    nc.tensor.matmul(
                        qps[:],
                        lhsT=xbt[:, c, xsl],
                        rhs=wq_sb[:, c, :],
                        start=(c == 0),
                        stop=(c == NCH - 1),
                    )
                qsb = qnorm.tile([P, Q_LOCAL], F32, name="qsb", tag="qsb")
                nc.vector.tensor_copy(qsb[:], qps[:])
                qg = qsb[:].rearrange("p (g d) -> p g d", g=2)
                sq = qnorm.tile([P, 2, D_HEAD], F32, name="qsq", tag="qsq")
                nc.vector.tensor_tensor(sq[:], qg, qg, ALU.mult)
                ss = qnorm.tile([P, 2], F32, name="qss", tag="qss")
                nc.vector.tensor_reduce(
                    ss[:], sq[:], axis=mybir.AxisListType.X, op=ALU.add
                )
                # rr = rsqrt(ss/64 + eps) via bit-trick seed + 2 Newton steps
                if rsqrt_act:
                    yrs = qnorm.tile([P, 2], F32, name="qrs", tag="qrs")
                    nc.scalar.activation(
                        yrs[:], ss[:], AF.Sqrt, bias=eps_t[:], scale=1.0 / D_HEAD
                    )
                    yact = qnorm.tile([P, 2], F32, name="qra", tag="qra")
                    nc.vector.reciprocal(yact[:], yrs[:])
                    y = yact[:]
                else:
                    ms = qnorm.tile([P, 2], F32, name="qms", tag="qms")
                    nc.vector.tensor_scalar(
                        ms[:], ss[:], 1.0 / D_HEAD, EPS, op0=ALU.mult, op1=ALU.add
                    )
                    xh = qnorm.tile([P, 2], F32, name="qxh", tag="qxh")
                    nc.vector.tensor_scalar(xh[:], ms[:], 0.5, None, op0=ALU.mult)
                    iy = qnorm.tile([P, 2], I32, name="qiy", tag="qiy")
                    nc.vector.tensor_scalar(
                        iy[:], ms[:].bitcast(I32), 1, None, op0=ALU.logical_shift_right
                    )
                    nc.vector.tensor_scalar(
                        iy[:], iy[:], -1, RSQRT_MAGIC, op0=ALU.mult, op1=ALU.add
                    )
                    y = iy[:].bitcast(F32)
                    for it in range(2):
                        y2 = qnorm.tile([P, 2], F32, name=f"qy2_{it}", tag=f"qy2_{it}")
                        nc.vector.tensor_tensor(y2[:], y, y, ALU.mult)
                        nc.vector.tensor_tensor(y2[:], y2[:], xh[:], ALU.mult)
                        nc.vector.tensor_scalar(
                            y2[:], y2[:], -1.0, 1.5, op0=ALU.mult, op1=ALU.add
                        )
                        yn = qnorm.tile([P, 2], F32, name=f"qyn_{it}", tag=f"qyn_{it}")
                        nc.vector.tensor_tensor(yn[:], y, y2[:], ALU.mult)
                        y = yn[:]
                q_hat = qnorm.tile([P, 2, D_HEAD], F32R, name="qhat", tag="qhat")
                nc.vector.tensor_tensor(
                    q_hat[:], qg, y[:, :, None].to_broadcast((P, 2, D_HEAD)), ALU.mult
                )
                for h in range(2):
                    pt = tpool.tile(
                        [D_HEAD, P], F32R, name="qpt", tag=tpool.name_tag
                    )
                    nc.tensor.transpose(pt[:], q_hat[:, h, :], ident_r[:])
                    nc.vector.tensor_copy(qhatT[h][:, ssl], pt[:])

        # ------------- phase 1: K/V projections + RMSNorm(k) + V' -------------
        with ExitStack() as p1:
            norm = p1.enter_context(tc.tile_pool(name="norm", bufs=6))
            qkps = p1.enter_context(tc.tile_pool(name="kvps", bufs=4, space="PSUM"))
            tps = p1.enter_context(tc.tile_pool(name="tps", bufs=4, space="PSUM"))

            class _P1Pool:
                name_tag = "kvps"

                @staticmethod
                def tile(shape, dt, name=None, tag=None):
                    return qkps.tile(shape, dt, name=name, tag="kvps")

            class _P1TPool:
                name_tag = "tps"

                @staticmethod
                def tile(shape, dt, name=None, tag=None):
                    return tps.tile(shape, dt, name=name, tag="tps")

            for sb in range(SB):
                ssl = bass.ts(sb, P)
                xbt = xball[sb // QC]
                xsl = bass.ts(sb % QC, P)

                kv_ps = qkps.tile([P, KV_LOCAL], F32, name="kv_ps", tag="kvps")
                for c in range(NCH):
                    nc.tensor.matmul(
                        kv_ps[:],
                        lhsT=xbt[:, c, xsl],
                        rhs=wkv_sb[:, c, :],
                        start=(c == 0),
                        stop=(c == NCH - 1),
                    )

                # RMSNorm stats for the 2 k heads
                k_ps = kv_ps[:, 0 : 2 * D_HEAD].rearrange("p (g d) -> p g d", g=2)
                sq = norm.tile([P, 2, D_HEAD], F32)
                nc.scalar.activation(sq[:], k_ps, AF.Square)
                ss = norm.tile([P, 2], F32)
                nc.vector.tensor_reduce(
                    ss[:], sq[:], axis=mybir.AxisListType.X, op=ALU.add
                )
                rs = norm.tile([P, 2], F32)
                nc.scalar.activation(
                    rs[:], ss[:], AF.Sqrt, bias=eps_t[:], scale=1.0 / D_HEAD
                )
                rr = norm.tile([P, 2], F32)
                nc.vector.reciprocal(rr[:], rs[:])

                k_hat = norm.tile([P, 2, D_HEAD], F32R)
                nc.vector.tensor_tensor(
                    k_hat[:],
                    k_ps,
                    rr[:, :, None].to_broadcast((P, 2, D_HEAD)),
                    ALU.mult,
                )

                # V chunks for both heads -> bf16 (ACT: same table as Square)
                nc.scalar.activation(
                    vp[:, :, sb, 0:D_HEAD],
                    kv_ps[:, 2 * D_HEAD : 4 * D_HEAD].rearrange(
                        "p (h d) -> p h d", h=2
                    ),
                    AF.Copy,
                )

                # transposes into [d, s]; wq*wk folded into the k side
                for h in range(2):
                    pt = tps.tile([D_HEAD, P], F32R, name="pt", tag="tps")
                    nc.tensor.transpose(pt[:], k_hat[:, h, :], ident_r[:])
                    if h == 0:
                        nc.scalar.activation(
                            khatT[h][:, ssl], pt[:], AF.Copy, scale=wkc_sb[:]
                        )
                    else:
                        nc.vector.tensor_scalar_mul(
                            khatT[h][:, ssl], pt[:], wkc_sb[:]
                        )

                # q side of the first q-tile, interleaved into the prefix
                # tail (ACT Sqrt: phase 1 owns the sqrt table)
                if sb >= 17 and (sb - 17) % 4 == 0:
                    emit_q_side_sb((sb - 17) // 4, _P1Pool, _P1TPool,
                                   rsqrt_act=True)



        # ---------- phase 2+3: attention with inlined O-projection ----------
        # PSUM banks: 2 score slots x3 banks, z accumulator 1 bank, shared
        # utility bank ("ops": O-proj / next-q-tile GEMM+transposes / ztp) = 8
        with ExitStack() as p2:
            spool = p2.enter_context(tc.tile_pool(name="sps", bufs=2, space="PSUM"))
            zqpool = p2.enter_context(tc.tile_pool(name="zqps", bufs=1, space="PSUM"))
            opool = p2.enter_context(tc.tile_pool(name="ops", bufs=1, space="PSUM"))
            ppool = p2.enter_context(tc.tile_pool(name="probs", bufs=3))
            znpool = p2.enter_context(tc.tile_pool(name="zn", bufs=3))
            rpool = p2.enter_context(tc.tile_pool(name="rcp", bufs=3))
            osb = p2.enter_context(tc.tile_pool(name="osb", bufs=3))

            class _P2Pool:
                name_tag = "ops"

                @staticmethod
                def tile(shape, dt, name=None, tag=None):
                    return opool.tile(shape, dt, name=name, tag="ops")

            def emit_oproj(qt, final=False):
                # final q-tile: S slots are free, so pipeline the matmuls
                # 2-wide through them and put half the copies on the idle ACT
                for sbl in range(QC):
                    sb = qt * QC + sbl
                    ot = osb.tile([P, D_MODEL], BF16, name="ot", tag="ot")
                    for half in range(2):
                        if final:
                            ops = spool.tile([P, QW], F32, name="ops", tag="sps")
                        else:
                            ops = opool.tile([P, QW], F32, name="ops", tag="ops")
                        nc.tensor.matmul(
                            ops[:],
                            lhsT=z_nT[:, bass.ts(sb, P)],
                            rhs=wot_sb[:, bass.ts(half, QW)],
                            start=True,
                            stop=True,
                        )
                        if final and half == 0:
                            nc.scalar.activation(
                                ot[:, bass.ts(half, QW)], ops[:], AF.Copy
                            )
                        else:
                            nc.vector.tensor_copy(ot[:, bass.ts(half, QW)], ops[:])
                    nc.sync.dma_start(out[bass.ts(sb, P), :], ot[:])

            for qt in range(QT):
                qsl = bass.ts(qt, QW)
                zn = znpool.tile([P, QC, P], BF16, name="zn", tag="zn")
                for h in range(HEADS_LOCAL):
                    zq = zqpool.tile([P, QC, D_HEAD + 1], F32, name="zq", tag="zq")
                    for kb0 in range(0, KB, EXP_BATCH):
                        nb = min(EXP_BATCH, KB - kb0)
                        sps = spool.tile(
                            [P, EXP_BATCH, QW], F32, name="sps", tag="sps"
                        )
                        for j in range(nb):
                            kb = kb0 + j
                            nc.tensor.matmul(
                                sps[:, j, :],
                                lhsT=khatT[h][:, bass.ts(kb, P)],
                                rhs=qhatT[h][:, qsl],
                                start=True,
                                stop=True,
                            )
                        probs = ppool.tile(
                            [P, EXP_BATCH, QW], BF16, name="probs", tag="probs"
                        )
                        nc.scalar.activation(
                            probs[:, 0:nb, :], sps[:, 0:nb, :], AF.Exp
                        )
                        # all 128 PV matmuls form ONE PSUM accumulation group
                        # (zq spans a single 2KB zero region): start marks the
                        # whole region pending-zero, each chunk's first touch
                        # overwrites, everything else accumulates
                        for j in range(nb):
                            kb = kb0 + j
                            for qc in range(QC):
                                nc.tensor.matmul(
                                    zq[:, qc, :],
                                    lhsT=probs[:, j, bass.ts(qc, P)],
                                    rhs=vp[:, h, kb, :],
                                    start=(kb == 0 and qc == 0),
                                    stop=(kb == KB - 1 and qc == QC - 1),
                                    skip_group_check=True,
                                )
                        # software-pipelined work emitted under the exp shadow:
                        # h0: O-projection of the previous q-tile
                        # h1: q side (GEMM+norm+transposes) of the next q-tile
                        if kb0 == EXP_BATCH and h == 0 and qt > 0:
                            emit_oproj(qt - 1)
                        if kb0 == EXP_BATCH and h == 1 and qt < QT - 1:
                            for sbl in range(QC):
                                emit_q_side_sb((qt + 1) * QC + sbl, _P2Pool, _P2Pool)
                    # normalize in q-major: z = z / rowsum (col 64)
                    rcp = rpool.tile([P, QC], F32, name="rcp", tag="rcp")
                    nc.vector.reciprocal(rcp[:], zq[:, :, D_HEAD])
                    nc.vector.tensor_tensor(
                        zn[:, :, bass.ts(h, D_HEAD)],
                        zq[:, :, 0:D_HEAD],
                        rcp[:, :, None].to_broadcast((P, QC, D_HEAD)),
                        ALU.mult,
                    )
                # transpose both heads at once into z_nT[d_local, s]
                ztp = opool.tile([P, QC, P], BF16, name="ztp", tag="ops")
                for qc in range(QC):
                    nc.tensor.transpose(ztp[:, qc, :], zn[:, qc, :], ident_b[:])
                    nc.vector.tensor_copy(
                        z_nT[:, qt * QW + qc * P : qt * QW + (qc + 1) * P],
                        ztp[:, qc, :],
                    )
            emit_oproj(QT - 1, final=True)

    if split_waits:
        _split_excess_waits(nc)
    return nc


def shard_inputs(x, Wqkv, bqkv, Wo, bo, wq, wk):
    import ml_dtypes

    x2 = np.ascontiguousarray(np.asarray(x, dtype=np.float32).reshape(SEQ, D_MODEL))
    Wqkv = np.asarray(Wqkv, dtype=np.float32)
    bqkv = np.asarray(bqkv, dtype=np.float32)
    Wo = np.asarray(Wo, dtype=np.float32)
    wq = np.asarray(wq, dtype=np.float32)
    wk = np.asarray(wk, dtype=np.float32)

    xta = np.zeros((DM_AUG, SEQ), np.float32)
    xta[:D_MODEL] = x2.T
    xta[D_MODEL] = 1.0
    xtb = np.ascontiguousarray(xta.astype(ml_dtypes.bfloat16))

    wkc = np.ascontiguousarray((wq * wk).reshape(D_HEAD, 1))

    in_maps = []
    for c in range(N_CORES):
        rows, brows = [], []
        for part in range(3):
            for h in (HEADS_LOCAL * c, HEADS_LOCAL * c + 1):
                sl = slice(part * D_MODEL + h * D_HEAD, part * D_MODEL + (h + 1) * D_HEAD)
                rows.append(Wqkv[sl])
                brows.append(bqkv[sl])
        Wl = np.concatenate(rows, 0)          # [384, 1024] rows [q0|q1|k0|k1|v0|v1]
        bl = np.concatenate(brows, 0)         # [384]
        wqkvta = np.zeros((DM_AUG, 384), np.float32)
        wqkvta[:D_MODEL] = Wl.T
        wqkvta[D_MODEL] = bl
        wkvt = np.ascontiguousarray(
            wqkvta[:, Q_LOCAL:].astype(ml_dtypes.bfloat16)
        )                                                              # [1152, 256]
        wqt = np.ascontiguousarray(
            wqkvta[:, :Q_LOCAL].astype(ml_dtypes.bfloat16)
        )                                                              # [1152, 128]
        cols = slice(HEADS_LOCAL * c * D_HEAD, (HEADS_LOCAL * c + HEADS_LOCAL) * D_HEAD)
        wotc = np.ascontiguousarray(Wo[:, cols].T.astype(ml_dtypes.bfloat16))
        in_maps.append(
            {
                "xtb": xtb,
                "wkvt": wkvt,
                "wqt": wqt,
                "wot": wotc,
                "wkc": wkc,
            }
        )
    return in_maps


_NC_CACHE = {}
LAST_RESULT = None


def kernel(x, Wqkv, bqkv, Wo, bo, wq, wk):
    import os
    from concourse.bass_utils import run_bass_kernel_spmd

    global LAST_RESULT
    assert np.asarray(x).shape == (1, SEQ, D_MODEL)
    in_maps = shard_inputs(x, Wqkv, bqkv, Wo, bo, wq, wk)
    if "nc" not in _NC_CACHE:
        _NC_CACHE["nc"] = build_core_kernel()
    nc = _NC_CACHE["nc"]
    trace = bool(int(os.environ.get("BASS_KERNEL_TRACE", "0")))
    res = run_bass_kernel_spmd(nc, in_maps, list(range(N_CORES)), trace=trace)
    LAST_RESULT = res
    acc = np.zeros((SEQ, D_MODEL), np.float64)
    for c in range(N_CORES):
        acc += res.results[c]["out"].astype(np.float64)
    acc += np.asarray(bo, dtype=np.float64)
    return acc.astype(np.float32).reshape(1, SEQ, D_MODEL)
